# revision 5
# baseline (speedup 1.0000x reference)
"""Multi-head attention with RoPE (B=32, N=577, C=768, H=12, D=64) on 8 TRN2 NeuronCores.

Strategy: data-parallel over batch (4 images per core), zero collectives.
Per-core layout: channels-on-partitions, tokens-on-free-dim for QKV/scores/proj.
  - w_qkv rows permuted on host so each head's q,k land as [32 even dims;
    32 odd dims] contiguously per head (head pair per 128-row tile). RoPE
    is then rot = A*C4 + swap32(A)*S4 where swap32 is four 32-row
    SBUF-to-SBUF DMAs; C4/S4 built on host (col 0 = identity for CLS).
  - scores_T[j, i] per head via one K=64 matmul; the two heads of a pair
    run concurrently in 64x128 PE-tiling mode (row_grp 0 / 64). Softmax
    without max-subtraction (scores*scale ~ N(0,1)); exp on ScalarE fused
    with the 1/sqrt(d) scale, output bf16.
  - v computed in [token, channel] layout with a ones-column per head.
  - attn@v reoriented: per (head pair, 128-query chunk) the exp tile is
    the stationary operand (M=query chunk) and v the moving operand
    (N=65: 64 dims + the ones column), accumulating over the 5 key tiles.
    Cost is 65 rows per matmul instead of 577 -- the PE's deep LDWEIGHTS
    pipeline hides the per-matmul stationary reload. psum col 64 is the
    softmax denominator for that (query, head); normalization is a DVE
    reciprocal of the two denominator columns plus two tensor_scalar
    multiplies into a [query, 128] bf16 staging tile.
  - the staging tile is DMA-transposed (HWDGE xbar, issued on the idle
    sync queue) into the [channel, token] activation tile consumed by
    the projection; the 65-token tail chunk is padded to 80 rows into a
    640-wide tile (garbage cols 577:592 never read).
  - projection/bias as before (bias via per-partition scalar add).
  - emission is a flat software pipeline over the 24 (image, head-pair)
    pairs: pair P's scores chased by pair P-1's attn@v chunks and
    transposes, with qkv/v units of image b+1 and projection columns of
    image b-1 as fillers; keep-warm dummies only where fillers run dry.
  - all matmuls bf16 with fp32 PSUM accumulation. Output stored bf16
    as [b, c, t]; host transposes back and widens to f32.
"""

import sys

sys.path.insert(0, "/opt/trn_rl_repo")

import numpy as np
import ml_dtypes

import concourse.bass as bass
import concourse.bacc as bacc
import concourse.tile as tile
from concourse import mybir
from concourse.bass_utils import run_bass_kernel_spmd

F32 = mybir.dt.float32
BF16 = mybir.dt.bfloat16

B, N, C = 32, 577, 768
H, D = 12, 64
NCORES = 8
BL = B // NCORES  # images per core
SCALE = D ** -0.5
NT = 5  # token tiles: 4*128 + 65
TWS = [128, 128, 128, 128, 65]
# free-dim chunks (psum-bank aligned)
NCH = [(0, 512), (512, 65)]
VCH = [(0, 512), (512, 256)]


def build(n_images=BL):
    nc = bacc.Bacc()
    xT = nc.declare_dram_parameter("xT", [n_images, C, N], BF16, isOutput=False)
    wqk = nc.declare_dram_parameter("wqk", [C, 2 * C], BF16, isOutput=False)
    wv = nc.declare_dram_parameter("wv", [C, C], BF16, isOutput=False)
    wp = nc.declare_dram_parameter("wp", [C, C], BF16, isOutput=False)
    c4d = nc.declare_dram_parameter("c4", [128, N], BF16, isOutput=False)
    s4d = nc.declare_dram_parameter("s4", [128, N], BF16, isOutput=False)
    bpd = nc.declare_dram_parameter("bproj", [6, 128], F32, isOutput=False)
    out = nc.declare_dram_parameter("out", [n_images, C, N], BF16, isOutput=True)

    Exp = mybir.ActivationFunctionType.Exp
    MUL = mybir.AluOpType.mult
    ADD = mybir.AluOpType.add
    NP = 6 * n_images  # total head-pair count across images

    with tile.TileContext(nc) as tc:
        with (
            tc.tile_pool(name="wpool", bufs=1) as wpool,
            tc.tile_pool(name="xp", bufs=2) as xp,
            tc.tile_pool(name="qkp", bufs=2) as qkp,
            tc.tile_pool(name="vp", bufs=2) as vp,
            tc.tile_pool(name="ep", bufs=3) as ep,
            tc.tile_pool(name="asp", bufs=2) as asp,
            tc.tile_pool(name="rp", bufs=2) as rp,
            tc.tile_pool(name="atp", bufs=2) as atp,
            tc.tile_pool(name="tp", bufs=3) as tp,
            tc.tile_pool(name="op", bufs=3) as op_,
            tc.tile_pool(name="ps", bufs=3, space="PSUM") as ps,
            tc.tile_pool(name="ps2", bufs=2, space="PSUM") as ps2,
            tc.tile_pool(name="psq", bufs=1, space="PSUM") as psq,
        ):
            # ---- qkv-critical loads spread over 3 DMA queues; PE warm-up
            # burst hides the DMA head ----
            qs = [nc.sync, nc.gpsimd, nc.scalar]
            wqk_sb = []
            wv_sb = []
            wp_sb = []

            def emit_x_loads(b):
                xsb = []
                for k in range(6):
                    t = xp.tile([128, N], BF16, tag=f"x{k}", name=f"x{k}")
                    qs[k % 3].dma_start(out=t[:], in_=xT[b, k * 128:(k + 1) * 128, :])
                    xsb.append(t)
                return xsb

            # interleave wqk and x0 per queue so the first qkv units can
            # start as soon as the lowest k-tiles land
            xsb0 = []
            for k in range(6):
                t = wpool.tile([128, 2 * C], BF16, tag=f"wqk{k}", name=f"wqk{k}")
                qs[k % 3].dma_start(out=t[:], in_=wqk[k * 128:(k + 1) * 128, :])
                wqk_sb.append(t)
                t = xp.tile([128, N], BF16, tag=f"x{k}", name=f"x{k}")
                qs[k % 3].dma_start(out=t[:], in_=xT[0, k * 128:(k + 1) * 128, :])
                xsb0.append(t)
            c4 = wpool.tile([128, N], BF16, tag="c4")
            nc.sync.dma_start(out=c4[:], in_=c4d[:])
            s4 = wpool.tile([128, N], BF16, tag="s4")
            nc.gpsimd.dma_start(out=s4[:], in_=s4d[:])
            # HAM warm-up: dummy matmuls on a memset tile while input DMAs
            # stream (~27us at the cold 1.2 GHz clock covers the DMA head)
            wu = wpool.tile([128, 512], BF16, tag="wu")
            nc.vector.memset(wu[:], 0.5)
            wups = ps.tile([128, 512], F32, tag="ps", name="wups")
            for _ in range(60):
                nc.tensor.matmul(out=wups[:, 0:512], lhsT=wu[:, 0:128], rhs=wu[:, 0:512],
                                 start=True, stop=True)

            def emit_dummy_mms(cnt):
                wd = ps.tile([128, 512], F32, tag="ps", name="wud")
                for _ in range(cnt):
                    nc.tensor.matmul(out=wd[:, 0:512], lhsT=wu[:, 0:128], rhs=wu[:, 0:512],
                                     start=True, stop=True)

            def load_wv():
                for k in range(6):
                    t = wpool.tile([128, C], BF16, tag=f"wv{k}", name=f"wv{k}")
                    qs[k % 3].dma_start(out=t[:], in_=wv[k * 128:(k + 1) * 128, :])
                    wv_sb.append(t)

            def load_rest():
                for k in range(6):
                    t = wpool.tile([128, C], BF16, tag=f"wp{k}", name=f"wp{k}")
                    qs[k % 3].dma_start(out=t[:], in_=wp[k * 128:(k + 1) * 128, :])
                    wp_sb.append(t)
                t = wpool.tile([128, 6], F32, tag="b")
                nc.sync.dma_start(out=t[:], in_=bpd[:].transpose([1, 0]))
                wp_sb.append(t)

            def emit_qkv_unit(xsb, qk_all, m):
                lhs_col = m * 128
                raw = tp.tile([128, N], BF16, tag="roperaw", name="raw")
                pqA = ps.tile([128, 512], F32, tag="ps", name="pqA")
                for k in range(6):
                    nc.tensor.matmul(
                        out=pqA[:, 0:512],
                        lhsT=wqk_sb[k][:, lhs_col:lhs_col + 128],
                        rhs=xsb[k][:, 0:512],
                        start=(k == 0), stop=(k == 5),
                    )
                nc.vector.tensor_copy(out=raw[:, 0:512], in_=pqA[:, 0:512])
                pqB = ps.tile([128, 512], F32, tag="ps", name="pqB")
                for k in range(6):
                    nc.tensor.matmul(
                        out=pqB[:, 0:65],
                        lhsT=wqk_sb[k][:, lhs_col:lhs_col + 128],
                        rhs=xsb[k][:, 512:N],
                        start=(k == 0), stop=(k == 5),
                    )
                nc.vector.tensor_copy(out=raw[:, 512:N], in_=pqB[:, 0:65])
                # RoPE: rot = A*C4 + pairswap(A)*S4   (col 0: c=1, s=0)
                sw = tp.tile([128, N], BF16, tag="ropesw", name="sw")
                nc.gpsimd.dma_start(out=sw[0:32, :], in_=raw[32:64, :])
                nc.sync.dma_start(out=sw[32:64, :], in_=raw[0:32, :])
                nc.gpsimd.dma_start(out=sw[64:96, :], in_=raw[96:128, :])
                nc.sync.dma_start(out=sw[96:128, :], in_=raw[64:96, :])
                tmp = tp.tile([128, N], BF16, tag="ropetmp", name="tmp")
                rot = tp.tile([128, N], BF16, tag="roperot", name="rot")
                nc.vector.tensor_tensor(out=tmp[:], in0=sw[:], in1=s4[:], op=MUL)
                nc.vector.tensor_tensor(out=rot[:], in0=raw[:], in1=c4[:], op=MUL)
                nc.vector.tensor_tensor(out=qk_all[:, m, :], in0=rot[:], in1=tmp[:], op=ADD)

            def emit_v_unit(xsb, v_all, t_i):
                tw = TWS[t_i]
                t0 = t_i * 128
                vdst = v_all[0:tw, t_i, :].rearrange("p (h c) -> p h c", c=65)
                pvA = ps.tile([128, 512], F32, tag="ps", name="pvA")
                for k in range(6):
                    nc.tensor.matmul(
                        out=pvA[0:tw, 0:512],
                        lhsT=xsb[k][:, t0:t0 + tw],
                        rhs=wv_sb[k][:, 0:512],
                        start=(k == 0), stop=(k == 5),
                    )
                nc.vector.tensor_copy(
                    out=vdst[:, 0:8, 0:64],
                    in_=pvA[0:tw, :].rearrange("p (h d) -> p h d", d=64),
                )
                pvB = ps.tile([128, 512], F32, tag="ps", name="pvB")
                for k in range(6):
                    nc.tensor.matmul(
                        out=pvB[0:tw, 0:256],
                        lhsT=xsb[k][:, t0:t0 + tw],
                        rhs=wv_sb[k][:, 512:768],
                        start=(k == 0), stop=(k == 5),
                    )
                nc.vector.tensor_copy(
                    out=vdst[:, 8:12, 0:64],
                    in_=pvB[0:tw, 0:256].rearrange("p (h d) -> p h d", d=64),
                )
                nc.vector.memset(vdst[:, 0:12, 64], 1.0)

            def emit_scores_j(qk_all, exps, m, j):
                jw = TWS[j]
                j0 = j * 128
                qt = qk_all[:, m, :]
                kt = qk_all[:, 6 + m, :]
                pscs = [ps2.tile([128, 1024], F32, tag="ps2", name="pscA"),
                        ps2.tile([128, 1024], F32, tag="ps2", name="pscB")]
                for c0, cw in NCH:
                    for hh in range(2):
                        r0, r1_ = hh * 64, hh * 64 + 64
                        nc.tensor.matmul(out=pscs[hh][0:jw, c0:c0 + cw], lhsT=kt[r0:r1_, j0:j0 + jw],
                                         rhs=qt[r0:r1_, c0:c0 + cw], start=True, stop=True)
                for hh in range(2):
                    nc.scalar.activation(out=exps[hh][0:jw, j, :], in_=pscs[hh][0:jw, 0:N],
                                         func=Exp, scale=SCALE)

            # ---- per-pair attn@v chase machinery ----
            pair_state = {}
            img_aT = {}
            v_img = {}
            qk_img = {}

            def chase_attnv(Q, qc):
                st = pair_state[Q]
                b, m = divmod(Q, 6)
                qw = TWS[qc]
                q0 = qc * 128
                if qc == 0:
                    st["a"] = asp.tile([128, 5, 128], BF16, tag="asb", name="asb")
                    # rows 65:80 of the last q-chunk are transposed as xbar
                    # padding -- initialize so the DMA never reads garbage
                    nc.vector.memset(st["a"][64:80, 4, :], 0.0)
                    st["r"] = rp.tile([128, 10], F32, tag="rsb", name="rsb")
                    if m == 0:
                        img_aT[b] = atp.tile([128, 6, 640], BF16, tag="aT", name="aT")
                if qc % 3 == 0:
                    st["ps"] = psq.tile([128, 512], F32, tag="psq", name="psq")
                pt = st["ps"]
                col0 = (qc % 3) * 130
                exps = st["exps"]
                v_all = v_img[b]
                # one open accumulation group per psum bank (2KB zero
                # region): the two heads run sequentially, not interleaved
                for hh in range(2):
                    h = 2 * m + hh
                    for j in range(NT):
                        jw = TWS[j]
                        rhs_v = v_all[0:jw, j, :].rearrange("p (h c) -> p h c", c=65)[:, h, :]
                        nc.tensor.matmul(
                            out=pt[0:qw, col0 + 65 * hh:col0 + 65 * hh + 65],
                            lhsT=exps[hh][0:jw, j, q0:q0 + qw],
                            rhs=rhs_v,
                            start=(j == 0), stop=(j == NT - 1),
                        )
                den = pt[0:qw, col0:col0 + 130].rearrange("p (a b) -> p a b", b=65)[:, :, 64]
                nc.vector.reciprocal(out=st["r"][0:qw, 2 * qc:2 * qc + 2], in_=den)
                for hh in range(2):
                    nc.vector.tensor_scalar(
                        out=st["a"][0:qw, qc, hh * 64:(hh + 1) * 64],
                        in0=pt[0:qw, col0 + 65 * hh:col0 + 65 * hh + 64],
                        scalar1=st["r"][0:qw, 2 * qc + hh:2 * qc + hh + 1],
                        scalar2=None,
                        op0=MUL,
                    )

            def chase_tr(Q, qc):
                st = pair_state[Q]
                b, m = divmod(Q, 6)
                qwp = 128 if qc < 4 else 80  # xbar needs p_dim % 16 == 0
                q0 = qc * 128
                nc.sync.dma_start(
                    out=img_aT[b][:, m, q0:q0 + qwp],
                    in_=st["a"][0:qwp, qc, :],
                    transpose=True,
                )

            def emit_proj_ct(b_img, ct):
                aT = img_aT[b_img]
                osb = op_.tile([128, N], BF16, tag="osb")
                ppA = ps.tile([128, 512], F32, tag="ps", name="ppA")
                for k in range(6):
                    nc.tensor.matmul(
                        out=ppA[:, 0:512],
                        lhsT=wp_sb[k][:, ct * 128:(ct + 1) * 128],
                        rhs=aT[:, k, 0:512],
                        start=(k == 0), stop=(k == 5),
                    )
                nc.scalar.add(out=osb[:, 0:512], in_=ppA[:, 0:512], add=bsb[:, ct:ct + 1])
                ppB = ps.tile([128, 512], F32, tag="ps", name="ppB")
                for k in range(6):
                    nc.tensor.matmul(
                        out=ppB[:, 0:65],
                        lhsT=wp_sb[k][:, ct * 128:(ct + 1) * 128],
                        rhs=aT[:, k, 512:N],
                        start=(k == 0), stop=(k == 5),
                    )
                nc.vector.tensor_scalar_add(out=osb[:, 512:N], in0=ppB[:, 0:65], scalar1=bsb[:, ct:ct + 1])
                eng = nc.sync if ct % 2 == 0 else nc.gpsimd
                eng.dma_start(out=out[b_img, ct * 128:(ct + 1) * 128, :], in_=osb[:])

            # ---- image 0 pre-phase: qkv + v while the warm-up burst and
            # input DMAs stream ----
            load_wv()
            qk0 = qkp.tile([128, 12, N], BF16, tag="qk", name="qk0")
            v0 = vp.tile([128, NT, 13 * 65], BF16, tag="v", name="v0")
            qk_img[0] = qk0
            v_img[0] = v0
            for m in range(4):
                emit_qkv_unit(xsb0, qk0, m)
            load_rest()
            bsb = wp_sb[6]
            del wp_sb[6:]
            for m in range(4, 12):
                emit_qkv_unit(xsb0, qk0, m)
            for t_i in range(NT):
                emit_v_unit(xsb0, v0, t_i)

            # ---- flat pair pipeline ----
            fq = []
            fq_done = 0
            pq_ = []

            for P in range(NP):
                b, m = divmod(P, 6)
                if m == 0:
                    fq = []
                    fq_done = 0
                    if b + 1 < n_images:
                        xsbn = emit_x_loads(b + 1)
                        qkn = qkp.tile([128, 12, N], BF16, tag="qk", name="qkn")
                        vn = vp.tile([128, NT, 13 * 65], BF16, tag="v", name="vn")
                        qk_img[b + 1] = qkn
                        v_img[b + 1] = vn
                        fq = [(lambda mm: (lambda: emit_qkv_unit(xsbn, qkn, mm)))(mm) for mm in range(12)] + \
                             [(lambda tt: (lambda: emit_v_unit(xsbn, vn, tt)))(tt) for tt in range(NT)]
                    pq_ = []
                    if b >= 1:
                        pq_ = [(lambda c: (lambda: emit_proj_ct(b - 1, c)))(c) for c in range(6)]
                exps = [
                    ep.tile([128, NT, N], BF16, tag="expA", name="expA"),
                    ep.tile([128, NT, N], BF16, tag="expB", name="expB"),
                ]
                pair_state[P] = {"exps": exps}
                for j in range(NT):
                    s = m * 5 + j
                    emit_scores_j(qk_img[b], exps, m, j)
                    if P >= 2 and j < 2:
                        chase_tr(P - 2, 3 + j)
                    if P >= 1:
                        chase_attnv(P - 1, j)
                        if j >= 2:
                            chase_tr(P - 1, j - 2)
                    emitted = False
                    if pq_ and s >= 12 and (s - 12) % 3 == 0:
                        pq_.pop(0)()
                        emitted = True
                    if fq:
                        target = ((s + 1) * 17 + 29) // 30
                        while fq and fq_done < target:
                            fq.pop(0)()
                            fq_done += 1
                            emitted = True
                    if not emitted and not fq and P >= NP - 6:
                        emit_dummy_mms(1)
                if P >= 2:
                    pair_state.pop(P - 2, None)

            # ---- tail: drain last pair + projection of the last image ----
            last = NP - 1
            chase_attnv(last, 0)
            chase_attnv(last, 1)
            chase_attnv(last, 2)
            chase_tr(last - 1, 3)
            chase_tr(last - 1, 4)
            chase_attnv(last, 3)
            chase_tr(last, 0)
            chase_attnv(last, 4)
            chase_tr(last, 1)
            emit_dummy_mms(2)
            chase_tr(last, 2)
            emit_dummy_mms(2)
            chase_tr(last, 3)
            chase_tr(last, 4)
            emit_dummy_mms(3)
            for ct in range(6):
                emit_proj_ct(n_images - 1, ct)
    nc.compile()
    return nc


def _qk_perm():
    """Row permutation of w_qkv's q,k sections -> head-interleaved pair-split."""
    perm = np.zeros(2 * C, dtype=np.int64)
    for m in range(12):
        sec = 0 if m < 6 else 1
        pair = m % 6
        base = m * 128
        hA, hB = 2 * pair, 2 * pair + 1
        perm[base + 0:base + 32] = sec * C + hA * D + 2 * np.arange(32)
        perm[base + 32:base + 64] = sec * C + hA * D + 2 * np.arange(32) + 1
        perm[base + 64:base + 96] = sec * C + hB * D + 2 * np.arange(32)
        perm[base + 96:base + 128] = sec * C + hB * D + 2 * np.arange(32) + 1
    return perm


def prep_inputs(x, w_qkv, w_proj, b_proj, cos, sin, n_images=BL):
    bf16 = ml_dtypes.bfloat16
    perm = _qk_perm()
    wqk = np.ascontiguousarray(w_qkv[perm, :].T).astype(bf16)  # [C, 2C]
    wv = np.ascontiguousarray(w_qkv[2 * C:3 * C, :].T).astype(bf16)  # [C, C]
    wp = np.ascontiguousarray(w_proj.T).astype(bf16)  # [C(in), C(out)]

    c4 = np.ones((128, N), dtype=np.float32)
    s4 = np.zeros((128, N), dtype=np.float32)
    p = np.arange(128)
    c4[:, 1:] = cos[:, p % 32].T
    s4[:, 1:] = sin[:, p % 32].T * np.where((p // 32) % 2 == 0, -1.0, 1.0)[:, None]
    c4 = c4.astype(bf16)
    s4 = s4.astype(bf16)

    bp = np.ascontiguousarray(b_proj.reshape(6, 128)).astype(np.float32)

    xT = np.ascontiguousarray(np.transpose(x, (0, 2, 1))).astype(bf16)  # [B, C, N]

    in_maps = []
    for i in range(NCORES):
        in_maps.append({
            "xT": xT[i * n_images:(i + 1) * n_images],
            "wqk": wqk, "wv": wv, "wp": wp,
            "c4": c4, "s4": s4, "bproj": bp,
        })
    return in_maps


_BUILT = {}


def kernel(x, w_qkv, w_proj, b_proj, cos, sin):
    x = np.asarray(x, dtype=np.float32)
    w_qkv = np.asarray(w_qkv, dtype=np.float32)
    w_proj = np.asarray(w_proj, dtype=np.float32)
    b_proj = np.asarray(b_proj, dtype=np.float32)
    cos = np.asarray(cos, dtype=np.float32)
    sin = np.asarray(sin, dtype=np.float32)

    if "nc" not in _BUILT:
        _BUILT["nc"] = build()
    nc = _BUILT["nc"]
    in_maps = prep_inputs(x, w_qkv, w_proj, b_proj, cos, sin)
    res = run_bass_kernel_spmd(nc, in_maps, core_ids=list(range(NCORES)))
    outs = np.concatenate([np.asarray(res.results[i]["out"]).astype(np.float32) for i in range(NCORES)], axis=0)
    return np.ascontiguousarray(np.transpose(outs, (0, 2, 1)))
